# revision 11
# baseline (speedup 1.0000x reference)
"""Trainium2 Bass kernel for nn_Conv_spe_12489764897428 (fp16 + xbar DMA-transpose).

Math: out[m, c] = sum_hw hs[0, c, h, w] * ms[m, 0, h, w]
  == matmul ms_flat[8, HW] @ hs_flat[191, HW].T with HW = 512*512 = 262144.

Sharding: HW (contraction) axis split across 8 cores; each core computes the
full [8, 191] partial over its 32768-wide HW slice; host sums the partials.

Design (vs the original fp32/f32r + PE-transpose pipeline, 1.40 ms/rep):
  - All device data in fp16 (inputs are randn; fp16 rounding gives ~1e-4
    rel err on the final reduction, gate is 2e-2). Halves HBM traffic.
  - hs channels padded 191 -> 192 on host (xbar wants rows % 16 == 0).
  - The xbar DMA-transpose streams each [192, 4096] hs chunk HBM->SBUF
    directly into [128hw, 32blk, 192ch] layout (probe-verified semantics:
    out[p, b, c] = in[c, b*128 + p]). No PE transposes, no PSUM staging,
    no DVE copies in the stream. All transposes stay on the SP HWDGE ring:
    alternating sync/scalar rings is ~13% faster but concurrent xbar
    transposes corrupt the data (measured rel err 0.44).
  - ms is pre-transposed on host into mst[p, b, m] = ms[m, b*128 + p]
    (0.5 MB/core). Per 128-hw block b one matmul accumulates into a single
    PSUM bank: psum[8, 192] += mst[:, b, :].T @ t[:, b, :]  (K=128 hw).
    Stationary is [128, 8] so the per-matmul reload is cheap (~100 ns/MM
    total incl. the N=192 stream; PE sits well under the DMA).
  - Final: one DVE copy psum -> SBUF, DMA [8, 191] fp32 partial out.

Measured (stub-timed repeat-delta, all 8 cores concurrent): ~66 us/rep =
~190 GB/s/core; the xbar transpose path is the binding constraint (plain
DMA measures ~388 GB/s, xbar ~233 GB/s single-core, less under 8-core
contention). 21x over the staged baseline's 1,404,766 ns.
"""

import numpy as np

import concourse.bass as bass
import concourse.mybir as mybir
import concourse.tile as tile
from concourse.bass_utils import run_bass_kernel_spmd
from concourse.masks import make_identity
from concourse.vector_clock import ScopedClock

N_CORES = 8
CH = 191                 # hs channels (band_hs)
CHP = 192                # padded to multiple of 16 for the xbar transpose
MB = 8                   # ms bands (band_ms)
HW = 512 * 512
HW_C = HW // N_CORES     # 32768 hw positions per core
W_CHUNK = 4096           # hw per transpose-DMA chunk
N_OUTER = HW_C // W_CHUNK
F32 = mybir.dt.float32
F16 = mybir.dt.float16

# ---------------------------------------------------------------------------
# Workarounds: walrus in this environment encodes at most ONE sync-wait per
# instruction (CTRL and S3_LW struct lowerings reject more with "Too many
# sync wait commands"). Tile freely attaches several. Split them: keep one
# wait on the instruction, hoist the rest onto same-engine NOPs placed just
# before it in the scheduled order.
# ---------------------------------------------------------------------------

_orig_lower_ordered_insts = tile.TileContext._lower_ordered_insts


def _split_multi_waits(nc, blocks):
    for bb, insts in list(blocks.items()):
        new_list = []
        changed = False
        for inst in insts:
            si = getattr(inst, "sync_info", None)
            waits = list(si.on_wait) if si is not None and si.on_wait else []
            if len(waits) > 1:
                si.on_wait = [waits[0]]
                for w in waits[1:]:
                    nop = mybir.InstNoOp(
                        name=nc.get_next_instruction_name(),
                        engine=inst.engine,
                        ins=[],
                        outs=[],
                        sync_info=mybir.SyncInfo(on_wait=[w], on_update=[]),
                        bass_nofuse=True,
                    )
                    nc.register_instruction(nop)
                    new_list.append(nop)
                changed = True
            new_list.append(inst)
        if changed:
            blocks[bb] = new_list


def _patched_lower_ordered_insts(self, postordered_blocks):
    _split_multi_waits(self.nc, postordered_blocks)
    return _orig_lower_ordered_insts(self, postordered_blocks)


tile.TileContext._lower_ordered_insts = _patched_lower_ordered_insts


def _patched_drain_and_barrier(self, tick_clock, wait_clock):
    nop_inst = self.nc.sync.nop(nofuse=True, hint="tail_drain_waits")
    wait_clock.add_sem_waits(
        nop_inst.ins, ScopedClock({None: tick_clock.global_clock})
    )
    si = nop_inst.ins.sync_info
    waits = list(si.on_wait) if si is not None and si.on_wait else []
    if len(waits) > 1:
        si.on_wait = [waits[0]]
        for w in waits[1:]:
            extra = self.nc.sync.nop(nofuse=True, hint="tail_drain_waits")
            esi = extra.ins.sync_info
            if esi is None:
                extra.ins.sync_info = mybir.SyncInfo(on_wait=[w], on_update=[])
            else:
                esi.on_wait = [w]

    self.nc.sync.drain()

    self.nc.all_engine_barrier()
    assert self.sems is not None
    popped = self.nc._tile_sem_poison_stack.pop()
    assert popped is self._sem_poison
    self.nc.clear_and_free_semaphores(list(self.sems.allocated().values()))
    self.nc.all_engine_barrier()


tile.TileContext._drain_and_barrier = _patched_drain_and_barrier


# ---------------------------------------------------------------------------
# Device kernel
# ---------------------------------------------------------------------------


def _emit_body(nc, pools, hs_d, mst_sb, out_d, ident, n_plain):
    hs_pool, h0_pool, h1_pool, pt_pool, tt_pool, pacc_pool, acc_pool = pools
    nb = W_CHUNK // 128
    pacc = pacc_pool.tile([MB, CHP], F32, tag="pacc")
    # Hybrid DMA: the last n_plain chunks bypass the xbar (which caps at
    # ~233 GB/s) via plain DMA on the ACT HWDGE ring (~388 GB/s) + PE
    # transposes; emitted FIRST so the transposes overlap the xbar stream.
    # Both paths produce the same hw = i*4096 + b*128 + p mapping, so mst
    # indexing is shared. NOTE: xbar transposes stay on ONE ring (SP) —
    # concurrent xbar transposes on both rings corrupt data (rel err 0.44).
    plain = list(range(N_OUTER - n_plain, N_OUTER))
    first = True
    for i in plain:
        h0 = h0_pool.tile([128, W_CHUNK], F16, tag="h0")
        h1 = h1_pool.tile([64, W_CHUNK], F16, tag="h1")
        nc.scalar.dma_start(out=h0, in_=hs_d[i][0:128, :])
        nc.scalar.dma_start(out=h1, in_=hs_d[i][128:192, :])
        tts = []
        for b in range(nb):
            pt = pt_pool.tile([128, CHP], F16, tag="pt")
            nc.tensor.transpose(
                pt[:, 0:128], h0[:, b * 128:(b + 1) * 128], ident)
            nc.tensor.transpose(
                pt[:, 128:192], h1[:, b * 128:(b + 1) * 128],
                ident[0:64, 0:64])
            tt = tt_pool.tile([128, CHP], F16, tag="tt")
            nc.vector.tensor_copy(tt, pt)
            tts.append(tt)
        for b in range(nb):
            nc.tensor.matmul(
                pacc, lhsT=mst_sb[:, i * nb + b, :], rhs=tts[b],
                start=first, stop=False)
            first = False
    for i in range(N_OUTER - n_plain):
        t = hs_pool.tile([128, nb, CHP], F16, tag="t")
        nc.sync.dma_start(out=t, in_=hs_d[i], transpose=True)
        for b in range(nb):
            nc.tensor.matmul(
                pacc,
                lhsT=mst_sb[:, i * nb + b, :],
                rhs=t[:, b, :],
                start=first,
                stop=(i == N_OUTER - n_plain - 1 and b == nb - 1),
            )
            first = False
    acc_sb = acc_pool.tile([MB, CHP], F32, tag="acc")
    nc.vector.tensor_copy(acc_sb, pacc)
    nc.sync.dma_start(out=out_d, in_=acc_sb[:, 0:CH])


def build_nc(reps=1, num_devices=N_CORES, bufs=6, timing_stub=False,
             n_plain=2):
    # timing_stub=True swaps the inputs for device-resident Internal tensors
    # (uninitialized): the identical instruction stream without the host->HBM
    # upload, for clean repeat-delta timing. kernel() never sets it.
    nc = bass.Bass("TRN2", target_bir_lowering=False, debug=False,
                   num_devices=num_devices)
    kind = "Internal" if timing_stub else "ExternalInput"
    hs_d = nc.dram_tensor("hs", [N_OUTER, CHP, W_CHUNK], F16, kind=kind).ap()
    mst_d = nc.dram_tensor("mst", [128, HW_C // 128, MB], F16, kind=kind).ap()
    out_d = nc.dram_tensor("out", [MB, CH], F32, kind="ExternalOutput").ap()
    nb = W_CHUNK // 128

    with tile.TileContext(nc) as tc:
        with (
            tc.tile_pool(name="singles", bufs=1) as singles,
            tc.tile_pool(name="hs", bufs=bufs) as hs_pool,
            tc.tile_pool(name="h0", bufs=2) as h0_pool,
            tc.tile_pool(name="h1", bufs=2) as h1_pool,
            tc.tile_pool(name="pt", bufs=4, space=bass.MemorySpace.PSUM) as pt_pool,
            tc.tile_pool(name="tt", bufs=nb) as tt_pool,
            tc.tile_pool(name="pacc", bufs=1, space=bass.MemorySpace.PSUM) as pacc_pool,
            tc.tile_pool(name="accp", bufs=1) as acc_pool,
        ):
            mst_sb = singles.tile([128, HW_C // 128, MB], F16)
            nc.sync.dma_start(out=mst_sb, in_=mst_d)
            ident = singles.tile([128, 128], F16)
            make_identity(nc, ident)
            pools = (hs_pool, h0_pool, h1_pool, pt_pool, tt_pool,
                     pacc_pool, acc_pool)
            if reps == 1:
                _emit_body(nc, pools, hs_d, mst_sb, out_d, ident, n_plain)
            else:
                with tc.For_i(0, reps, 1) as _i:
                    _emit_body(nc, pools, hs_d, mst_sb, out_d, ident, n_plain)
    return nc


# ---------------------------------------------------------------------------
# Host wrapper
# ---------------------------------------------------------------------------

_NC_CACHE = {}


def _get_nc(**kwargs):
    key = tuple(sorted(kwargs.items()))
    if key not in _NC_CACHE:
        _NC_CACHE[key] = build_nc(**kwargs)
    return _NC_CACHE[key]


def make_in_maps(hs, ms):
    hsf = np.asarray(hs, dtype=np.float32).reshape(CH, HW).astype(np.float16)
    msf = np.asarray(ms, dtype=np.float32).reshape(MB, HW).astype(np.float16)
    hsp = np.zeros((CHP, HW), dtype=np.float16)
    hsp[:CH] = hsf
    in_maps = []
    for c in range(N_CORES):
        sl = slice(c * HW_C, (c + 1) * HW_C)
        # chunk-major layout: hs_c[i, ch, w] = hsp[ch, c*HW_C + i*W_CHUNK + w]
        hs_c = np.ascontiguousarray(
            hsp[:, sl].reshape(CHP, N_OUTER, W_CHUNK).transpose(1, 0, 2))
        # mst[p, b, m] = ms[m, b*128 + p]  (xbar-transpose block order)
        mst_c = np.ascontiguousarray(
            msf[:, sl].reshape(MB, HW_C // 128, 128).transpose(2, 1, 0))
        in_maps.append({"hs": hs_c, "mst": mst_c})
    return in_maps


def kernel(hs, ms):
    in_maps = make_in_maps(hs, ms)
    nc = _get_nc()
    res = run_bass_kernel_spmd(nc, in_maps, list(range(N_CORES)))
    out = np.zeros((MB, CH), np.float64)
    for c in range(N_CORES):
        out += res.results[c]["out"].astype(np.float64)
    return out.astype(np.float32)[:, :, None, None]


# revision 12
# speedup vs baseline: 1.2986x; 1.2986x over previous
"""Trainium2 Bass kernel for nn_Conv_spe_12489764897428 (fp16 + xbar DMA-transpose).

Math: out[m, c] = sum_hw hs[0, c, h, w] * ms[m, 0, h, w]
  == matmul ms_flat[8, HW] @ hs_flat[191, HW].T with HW = 512*512 = 262144.

Sharding: HW (contraction) axis split across 8 cores; each core computes the
full [8, 191] partial over its 32768-wide HW slice; host sums the partials.

Design (vs the original fp32/f32r + PE-transpose pipeline, 1.40 ms/rep):
  - All device data in fp16 (inputs are randn; fp16 rounding gives ~1e-4
    rel err on the final reduction, gate is 2e-2). Halves HBM traffic.
  - hs channels padded 191 -> 192 on host (xbar wants rows % 16 == 0).
  - The xbar DMA-transpose streams each [192, 4096] hs chunk HBM->SBUF
    directly into [128hw, 32blk, 192ch] layout (probe-verified semantics:
    out[p, b, c] = in[c, b*128 + p]). No PE transposes, no PSUM staging,
    no DVE copies in the stream. All transposes stay on the SP HWDGE ring:
    alternating sync/scalar rings is ~13% faster but concurrent xbar
    transposes corrupt the data (measured rel err 0.44).
  - ms is pre-transposed on host into mst[p, b, m] = ms[m, b*128 + p]
    (0.5 MB/core). Per 128-hw block b one matmul accumulates into a single
    PSUM bank: psum[8, 192] += mst[:, b, :].T @ t[:, b, :]  (K=128 hw).
    Stationary is [128, 8] so the per-matmul reload is cheap (~100 ns/MM
    total incl. the N=192 stream; PE sits well under the DMA).
  - Final: one DVE copy psum -> SBUF, DMA [8, 191] fp32 partial out.

Measured (stub-timed repeat-delta, all 8 cores concurrent): ~66 us/rep =
~190 GB/s/core; the xbar transpose path is the binding constraint (plain
DMA measures ~388 GB/s, xbar ~233 GB/s single-core, less under 8-core
contention). 21x over the staged baseline's 1,404,766 ns.
"""

import numpy as np

import concourse.bass as bass
import concourse.mybir as mybir
import concourse.tile as tile
from concourse.bass_utils import run_bass_kernel_spmd
from concourse.vector_clock import ScopedClock

N_CORES = 8
CH = 191                 # hs channels (band_hs)
CHP = 192                # padded to multiple of 16 for the xbar transpose
MB = 8                   # ms bands (band_ms)
HW = 512 * 512
HW_C = HW // N_CORES     # 32768 hw positions per core
W_CHUNK = 8192           # hw per transpose-DMA chunk (c8192 measured fastest)
N_OUTER = HW_C // W_CHUNK
F32 = mybir.dt.float32
F16 = mybir.dt.float16

# ---------------------------------------------------------------------------
# Workarounds: walrus in this environment encodes at most ONE sync-wait per
# instruction (CTRL and S3_LW struct lowerings reject more with "Too many
# sync wait commands"). Tile freely attaches several. Split them: keep one
# wait on the instruction, hoist the rest onto same-engine NOPs placed just
# before it in the scheduled order.
# ---------------------------------------------------------------------------

_orig_lower_ordered_insts = tile.TileContext._lower_ordered_insts


def _split_multi_waits(nc, blocks):
    for bb, insts in list(blocks.items()):
        new_list = []
        changed = False
        for inst in insts:
            si = getattr(inst, "sync_info", None)
            waits = list(si.on_wait) if si is not None and si.on_wait else []
            if len(waits) > 1:
                si.on_wait = [waits[0]]
                for w in waits[1:]:
                    nop = mybir.InstNoOp(
                        name=nc.get_next_instruction_name(),
                        engine=inst.engine,
                        ins=[],
                        outs=[],
                        sync_info=mybir.SyncInfo(on_wait=[w], on_update=[]),
                        bass_nofuse=True,
                    )
                    nc.register_instruction(nop)
                    new_list.append(nop)
                changed = True
            new_list.append(inst)
        if changed:
            blocks[bb] = new_list


def _patched_lower_ordered_insts(self, postordered_blocks):
    _split_multi_waits(self.nc, postordered_blocks)
    return _orig_lower_ordered_insts(self, postordered_blocks)


tile.TileContext._lower_ordered_insts = _patched_lower_ordered_insts


def _patched_drain_and_barrier(self, tick_clock, wait_clock):
    nop_inst = self.nc.sync.nop(nofuse=True, hint="tail_drain_waits")
    wait_clock.add_sem_waits(
        nop_inst.ins, ScopedClock({None: tick_clock.global_clock})
    )
    si = nop_inst.ins.sync_info
    waits = list(si.on_wait) if si is not None and si.on_wait else []
    if len(waits) > 1:
        si.on_wait = [waits[0]]
        for w in waits[1:]:
            extra = self.nc.sync.nop(nofuse=True, hint="tail_drain_waits")
            esi = extra.ins.sync_info
            if esi is None:
                extra.ins.sync_info = mybir.SyncInfo(on_wait=[w], on_update=[])
            else:
                esi.on_wait = [w]

    self.nc.sync.drain()

    self.nc.all_engine_barrier()
    assert self.sems is not None
    popped = self.nc._tile_sem_poison_stack.pop()
    assert popped is self._sem_poison
    self.nc.clear_and_free_semaphores(list(self.sems.allocated().values()))
    self.nc.all_engine_barrier()


tile.TileContext._drain_and_barrier = _patched_drain_and_barrier


# ---------------------------------------------------------------------------
# Device kernel
# ---------------------------------------------------------------------------


def _emit_body(nc, pools, hs_d, mst_sb, out_d):
    hs_pool, pacc_pool, acc_pool = pools
    nb = W_CHUNK // 128
    pacc = pacc_pool.tile([MB, CHP], F32, tag="pacc")
    for i in range(N_OUTER):
        t = hs_pool.tile([128, nb, CHP], F16, tag="t")
        # NOTE: keep all transposes on ONE HWDGE ring (SP). Alternating
        # sync/scalar measures ~13% faster but concurrent xbar transposes
        # corrupt the data (rel err 0.44) — the xbar S2M is shared state.
        nc.sync.dma_start(out=t, in_=hs_d[i], transpose=True)
        for b in range(nb):
            nc.tensor.matmul(
                pacc,
                lhsT=mst_sb[:, i * nb + b, :],
                rhs=t[:, b, :],
                start=(i == 0 and b == 0),
                stop=(i == N_OUTER - 1 and b == nb - 1),
            )
    acc_sb = acc_pool.tile([MB, CHP], F32, tag="acc")
    nc.vector.tensor_copy(acc_sb, pacc)
    nc.sync.dma_start(out=out_d, in_=acc_sb[:, 0:CH])


def build_nc(reps=1, num_devices=N_CORES, bufs=3, timing_stub=False):
    # timing_stub=True swaps the inputs for device-resident Internal tensors
    # (uninitialized): the identical instruction stream without the host->HBM
    # upload, for clean repeat-delta timing. kernel() never sets it.
    nc = bass.Bass("TRN2", target_bir_lowering=False, debug=False,
                   num_devices=num_devices)
    kind = "Internal" if timing_stub else "ExternalInput"
    hs_d = nc.dram_tensor("hs", [N_OUTER, CHP, W_CHUNK], F16, kind=kind).ap()
    mst_d = nc.dram_tensor("mst", [128, HW_C // 128, MB], F16, kind=kind).ap()
    out_d = nc.dram_tensor("out", [MB, CH], F32, kind="ExternalOutput").ap()

    with tile.TileContext(nc) as tc:
        with (
            tc.tile_pool(name="singles", bufs=1) as singles,
            tc.tile_pool(name="hs", bufs=bufs) as hs_pool,
            tc.tile_pool(name="pacc", bufs=1, space=bass.MemorySpace.PSUM) as pacc_pool,
            tc.tile_pool(name="accp", bufs=1) as acc_pool,
        ):
            mst_sb = singles.tile([128, HW_C // 128, MB], F16)
            nc.sync.dma_start(out=mst_sb, in_=mst_d)
            pools = (hs_pool, pacc_pool, acc_pool)
            if reps == 1:
                _emit_body(nc, pools, hs_d, mst_sb, out_d)
            else:
                with tc.For_i(0, reps, 1) as _i:
                    _emit_body(nc, pools, hs_d, mst_sb, out_d)
    return nc


# ---------------------------------------------------------------------------
# Host wrapper
# ---------------------------------------------------------------------------

_NC_CACHE = {}


def _get_nc(**kwargs):
    key = tuple(sorted(kwargs.items()))
    if key not in _NC_CACHE:
        _NC_CACHE[key] = build_nc(**kwargs)
    return _NC_CACHE[key]


def make_in_maps(hs, ms):
    hsf = np.asarray(hs, dtype=np.float32).reshape(CH, HW).astype(np.float16)
    msf = np.asarray(ms, dtype=np.float32).reshape(MB, HW).astype(np.float16)
    hsp = np.zeros((CHP, HW), dtype=np.float16)
    hsp[:CH] = hsf
    in_maps = []
    for c in range(N_CORES):
        sl = slice(c * HW_C, (c + 1) * HW_C)
        # chunk-major layout: hs_c[i, ch, w] = hsp[ch, c*HW_C + i*W_CHUNK + w]
        hs_c = np.ascontiguousarray(
            hsp[:, sl].reshape(CHP, N_OUTER, W_CHUNK).transpose(1, 0, 2))
        # mst[p, b, m] = ms[m, b*128 + p]  (xbar-transpose block order)
        mst_c = np.ascontiguousarray(
            msf[:, sl].reshape(MB, HW_C // 128, 128).transpose(2, 1, 0))
        in_maps.append({"hs": hs_c, "mst": mst_c})
    return in_maps


def kernel(hs, ms):
    in_maps = make_in_maps(hs, ms)
    nc = _get_nc()
    res = run_bass_kernel_spmd(nc, in_maps, list(range(N_CORES)))
    out = np.zeros((MB, CH), np.float64)
    for c in range(N_CORES):
        out += res.results[c]["out"].astype(np.float64)
    return out.astype(np.float32)[:, :, None, None]
